# revision 1
# baseline (speedup 1.0000x reference)
"""Causal multi-head attention (B=2, S=2048, D=1024, H=16) on 8 TRN2 NeuronCores.

Sharding: core c handles batch b=c//4 and the 4 heads [4*(c%4), 4*(c%4)+4).
Each core computes its heads' Q/K/V projections, causal attention, and a
column-shard of the output projection; the host sums the 4 partials per batch
and adds bo.

On-device layout strategy (zero on-device transposes):
  - host supplies xT [D,S] and pre-transposed weights
  - qT,kT computed as [e,s] (head-dim on partitions), v as [s,e]
  - scores computed transposed: scoresT[k,q] = kT^T @ qT  (softmax over the
    partition dim; no max-subtraction needed -- scores are O(1) here)
  - causal mask = 0/1 multiply on probs (GpSimd, otherwise idle)
  - AV stationary carries a 65th ones-column -> row 64 of wv psum is the
    softmax denominator (PE partition-reduction for free)
  - AV: wvT[e,q] = v^T @ probsT  -> exactly the moving-operand layout the
    O-projection needs; output written transposed, un-transposed on host.
All matmuls run in float32r (full PE rate at N>=256, ~1.5e-4 rel err).
"""

import numpy as np

B, S, D, H = 2, 2048, 1024, 16
HD = D // H  # 64
NCORES = 8
P = 128
SB = 512          # s/q block size
NSB = S // SB     # 4
NDK = D // P      # 8
NKT_ALL = S // P  # 16

_CACHE = {}


def _build_nc():
    import concourse.bacc as bacc
    import concourse.mybir as mybir
    import concourse.tile as tile

    F32R, F32 = mybir.dt.float32r, mybir.dt.float32
    EXP = mybir.ActivationFunctionType.Exp
    ADD = mybir.AluOpType.add
    MULT = mybir.AluOpType.mult

    nc = bacc.Bacc(None)
    xT = nc.declare_dram_parameter("xT", [D, S], F32R, isOutput=False)
    wqk = nc.declare_dram_parameter("wqkT", [D, 512], F32R, isOutput=False)
    wv = nc.declare_dram_parameter("wvT", [D, 256], F32R, isOutput=False)
    wo = nc.declare_dram_parameter("woT", [256, D], F32R, isOutput=False)
    bqk = nc.declare_dram_parameter("bqk", [P, 4], F32, isOutput=False)
    bv = nc.declare_dram_parameter("bv", [1, 256], F32, isOutput=False)
    masks = nc.declare_dram_parameter("masks", [P, 4 * SB], F32, isOutput=False)
    outT = nc.declare_dram_parameter("outT", [D, S], F32, isOutput=True)

    with tile.TileContext(nc) as tc:
        with (
            tc.tile_pool(name="w", bufs=1) as wp,
            tc.tile_pool(name="x", bufs=2) as xp,
            tc.tile_pool(name="qk", bufs=1) as qkp,
            tc.tile_pool(name="pb", bufs=3) as pbp,
            tc.tile_pool(name="sm", bufs=2) as smp,
            tc.tile_pool(name="o", bufs=4) as op_,
        ):
            # ---- weights / constants (DMA order = need order) ----
            wqk_r = wqk[:].rearrange("(dk p) m -> p dk m", p=P)
            wqk_sb = wp.tile([P, NDK, 512], F32R)
            xT_r = xT[:].rearrange("(dk p) s -> p dk s", p=P)
            x_tiles = []
            x0 = xp.tile([P, NDK, SB], F32R, tag="x", name="x0")
            for d0 in range(0, NDK, 2):
                nc.sync.dma_start(wqk_sb[:, d0:d0 + 2, :], wqk_r[:, d0:d0 + 2, :])
                nc.sync.dma_start(x0[:, d0:d0 + 2, :], xT_r[:, d0:d0 + 2, 0:SB])
            x_tiles.append(x0)
            wv_sb = wp.tile([P, NDK, 256], F32R)
            nc.sync.dma_start(wv_sb[:], wv[:].rearrange("(dk p) m -> p dk m", p=P))
            bqk_sb = wp.tile([P, 4], F32)
            nc.sync.dma_start(bqk_sb[:], bqk[:])
            bv_sb = wp.tile([1, 256], F32)
            nc.sync.dma_start(bv_sb[:], bv[:])
            bv_bc = wp.tile([P, 256], F32)
            nc.gpsimd.partition_broadcast(bv_bc[:], bv_sb[:])
            mask_sb = wp.tile([P, 4, SB], F32)
            nc.sync.dma_start(mask_sb[:], masks[:].rearrange("p (t c) -> p t c", t=4))
            wo_sb = wp.tile([P, 2, D], F32R)
            nc.sync.dma_start(wo_sb[:], wo[:].rearrange("(k p) m -> p k m", p=P))

            # ---- persistent activations ----
            qT = [qkp.tile([P, S], F32R, tag=f"qT{p}", name=f"qT{p}") for p in range(2)]
            kT = [qkp.tile([P, S], F32R, tag=f"kT{p}", name=f"kT{p}") for p in range(2)]
            v_sb = qkp.tile([P, NKT_ALL, 4, HD + 1], F32R, tag="v")
            nc.vector.memset(v_sb[:, :, :, HD:HD + 1].bitcast(F32), 1.0)
            zT_all = qkp.tile([P, 2, NSB, SB], F32R, tag="zT")

            with (
                tc.tile_pool(name="psS", bufs=2, space="PSUM") as psS,   # sc = 4 banks
                tc.tile_pool(name="psW", bufs=1, space="PSUM") as psW,   # wv0+wv1 = 2 banks
                tc.tile_pool(name="psG", bufs=2, space="PSUM") as psG,   # proj = 2 banks
            ):
                for blk in range(NSB):
                    # ======== projections for s-block blk ========
                    if blk + 1 < NSB:  # prefetch next x block
                        xn = xp.tile([P, NDK, SB], F32R, tag="x", name=f"x{blk+1}")
                        nc.sync.dma_start(xn[:], xT_r[:, :, (blk + 1) * SB:(blk + 2) * SB])
                        x_tiles.append(xn)
                    x_sb = x_tiles[blk]
                    for p in range(2):
                        for t in range(2):  # 0 = q, 1 = k
                            ps = psG.tile([P, SB], F32, tag="g")
                            c0 = 256 * t + 128 * p
                            for dk in range(NDK):
                                nc.tensor.matmul(
                                    ps[:], wqk_sb[:, dk, c0:c0 + 128], x_sb[:, dk, :],
                                    start=(dk == 0), stop=(dk == NDK - 1))
                            dst = (qT if t == 0 else kT)[p]
                            nc.vector.tensor_scalar_add(
                                dst[:, blk * SB:(blk + 1) * SB], ps[:],
                                bqk_sb[:, 2 * t + p: 2 * t + p + 1])
                    for st in range(4):
                        kt = blk * 4 + st
                        psv = psG.tile([P, 256], F32, tag="g")
                        for dk in range(NDK):
                            nc.tensor.matmul(
                                psv[:], x_sb[:, dk, st * P:(st + 1) * P], wv_sb[:, dk, :],
                                start=(dk == 0), stop=(dk == NDK - 1))
                        nc.vector.tensor_tensor(
                            v_sb[:, kt, :, 0:HD],
                            psv[:].rearrange("p (h e) -> p h e", h=4),
                            bv_bc[:].rearrange("p (h e) -> p h e", h=4), ADD)

                    # ======== attention for q-block j = blk ========
                    j = blk
                    NKT = 4 * (j + 1)
                    for p in range(2):
                        wv_tiles = [psW.tile([P, SB], F32, tag=f"wv{half}",
                                             name=f"wv{half}") for half in range(2)]
                        for kt in range(NKT):
                            # both halves' scores adjacent -> PE row-group overlap
                            sc = psS.tile([P, 2, SB], F32, tag="sc")
                            for half in range(2):
                                base = 64 * half
                                nc.tensor.matmul(
                                    sc[:, half, :],
                                    kT[p][base:base + 64, kt * P:(kt + 1) * P],
                                    qT[p][base:base + 64, j * SB:(j + 1) * SB],
                                    start=True, stop=True)
                            t = kt - (NKT - 4)
                            if t >= 0:  # additive causal mask on scores (DVE)
                                nc.vector.tensor_tensor(
                                    sc[:], sc[:],
                                    mask_sb[:, t, None, :].to_broadcast([P, 2, SB]),
                                    ADD)
                            pr = pbp.tile([P, 2, SB], F32R, tag="pr")
                            nc.scalar.activation(pr[:], sc[:], EXP)
                            for half in range(2):
                                nc.tensor.matmul(
                                    wv_tiles[half][0:HD + 1, :],
                                    v_sb[:, kt, 2 * p + half, :], pr[:, half, :],
                                    start=(kt == 0), stop=(kt == NKT - 1))
                        # quick-release wv psum -> sbuf, then normalize from sbuf
                        for half in range(2):
                            wv_ps = wv_tiles[half]
                            wvs = smp.tile([HD + 1, SB], F32, tag="wvs", name=f"wvs{half}")
                            nc.vector.tensor_copy(wvs[:], wv_ps[0:HD + 1, :])
                            dn = smp.tile([1, SB], F32, tag="dn")
                            nc.vector.tensor_copy(dn[:], wv_ps[HD:HD + 1, :])
                            rb = smp.tile([P, SB], F32, tag="rb")
                            nc.gpsimd.partition_broadcast(rb[0:HD, :], dn[:])
                            rr = smp.tile([P, SB], F32, tag="rr")
                            rs = smp.tile([P, SB], F32, tag="rs")
                            nc.vector.reciprocal_approx_accurate(
                                rr[0:HD, :], rb[0:HD, :], rs[0:HD, :])
                            if half == 0:
                                nc.vector.tensor_tensor(
                                    zT_all[0:HD, p, j, :], wvs[0:HD, :], rr[0:HD, :],
                                    MULT)
                            else:
                                zt_o = smp.tile([HD, SB], F32R, tag="zt_o")
                                nc.vector.tensor_tensor(
                                    zt_o[:], wvs[0:HD, :], rr[0:HD, :], MULT)
                                nc.sync.dma_start(zT_all[HD:P, p, j, :], zt_o[:])

            # ======== output projection (stationary reused across q-blocks) ====
            with tc.tile_pool(name="psO", bufs=4, space="PSUM") as psO:
                for et in range(NDK):
                    pos = [psO.tile([P, SB], F32, tag="po", name=f"po{j}")
                           for j in range(NSB)]
                    for k2 in range(2):
                        for j in range(NSB):
                            nc.tensor.matmul(
                                pos[j][:], wo_sb[:, k2, et * P:(et + 1) * P],
                                zT_all[:, k2, j, :],
                                start=(k2 == 0), stop=(k2 == 1))
                    for j in range(NSB):
                        ot = op_.tile([P, SB], F32, tag="ot")
                        nc.any.tensor_copy(ot[:], pos[j][:])
                        nc.sync.dma_start(
                            outT[:][et * P:(et + 1) * P, j * SB:(j + 1) * SB], ot[:])

    nc.compile()
    return nc


def _host_inputs(inputs, Wq, bq, Wk, bk, Wv, bv, Wo, bo):
    """Build the 8 per-core input maps."""
    scale = np.float32(1.0 / np.sqrt(HD))
    in_maps = []
    # causal 0/1 masks for the 4 diagonal k-tiles of a q-block
    kk = np.arange(P)[:, None]
    qq = np.arange(SB)[None, :]
    m = np.zeros((P, 4 * SB), dtype=np.float32)
    for t in range(4):
        m[:, t * SB:(t + 1) * SB] = np.where(P * t + kk <= qq, 0.0, -1e30)
    for c in range(NCORES):
        b = c // 4
        hg = c % 4
        hs = slice(4 * hg, 4 * hg + 4)
        xTc = np.ascontiguousarray(np.asarray(inputs[b], np.float32).T)
        WqT = np.asarray(Wq[hs], np.float32).transpose(2, 0, 1).reshape(D, 256) * scale
        WkT = np.asarray(Wk[hs], np.float32).transpose(2, 0, 1).reshape(D, 256)
        WvT = np.asarray(Wv[hs], np.float32).transpose(2, 0, 1).reshape(D, 256)
        wqkT = np.ascontiguousarray(np.concatenate([WqT, WkT], axis=1))
        bq_c = np.asarray(bq[hs], np.float32).reshape(256) * scale
        bk_c = np.asarray(bk[hs], np.float32).reshape(256)
        bqk_c = np.stack([bq_c[0:128], bq_c[128:256], bk_c[0:128], bk_c[128:256]], axis=1)
        bv_c = np.asarray(bv[hs], np.float32).reshape(1, 256)
        woT = np.ascontiguousarray(np.asarray(Wo, np.float32)[:, 256 * hg:256 * (hg + 1)].T)
        in_maps.append({
            "xT": xTc, "wqkT": wqkT, "wvT": np.ascontiguousarray(WvT),
            "woT": woT, "bqk": np.ascontiguousarray(bqk_c), "bv": bv_c,
            "masks": m,
        })
    return in_maps


def kernel(inputs, Wq, bq, Wk, bk, Wv, bv, Wo, bo):
    from concourse.bass_utils import run_bass_kernel_spmd

    if "nc" not in _CACHE:
        _CACHE["nc"] = _build_nc()
    nc = _CACHE["nc"]
    in_maps = _host_inputs(inputs, Wq, bq, Wk, bk, Wv, bv, Wo, bo)
    res = run_bass_kernel_spmd(nc, in_maps, list(range(NCORES)))
    out = np.zeros((B, S, D), dtype=np.float32)
    for c in range(NCORES):
        out[c // 4] += res.results[c]["outT"].T
    out += np.asarray(bo, np.float32)[None, None, :]
    return out



# revision 14
# speedup vs baseline: 1.3189x; 1.3189x over previous
"""Causal multi-head attention (B=2, S=2048, D=1024, H=16) on 8 TRN2 NeuronCores.

Sharding: core c handles batch b=c//4 and the 4 heads [4*(c%4), 4*(c%4)+4).
Each core computes its heads' Q/K/V projections, causal attention, and a
column-shard of the output projection; the host sums the 4 partials per batch
and adds bo.

Key layout/perf choices (v2):
  - all weights/activations bf16 except: score psums f32, q/k stored fp8e4m3
  - scores run as fp8 DoubleRow matmuls: q/k stored [32h+r, t, s] with the
    64-dim head split into two 32-row groups (t); the projection weight
    columns are pre-permuted on the host so the PSUM partition layout matches
    (bias-add stays partition-preserving). 2x PE throughput on scores.
  - diagonal k-tiles narrowed: score/exp/AV only touch q >= 128*t, the
    causal triangle mask is a single [128,128] additive tile.
  - AV stationary carries a ones-column; half 0 outputs psum rows 0..64
    (row 64 = denominator), half 1 uses [one, v] so its 65 output rows land
    at 63..127 (row 63 = denominator) -> z lands on its natural partitions,
    no cross-partition DMA.
  - normalization: reciprocal on the single denominator row, gpsimd
    partition-broadcast, one DVE mult into bf16 zT.
  - O-projection for block j is issued one block late (lowest priority) so
    it fills PE gaps; exp is the only Scalar-engine op (one act table).
"""

import numpy as np

B, S, D, H = 2, 2048, 1024, 16
HD = D // H  # 64
NCORES = 8
P = 128
SB = 512          # s/q block size
NSB = S // SB     # 4
NDK = D // P      # 8
NKT_ALL = S // P  # 16

_CACHE = {}


def _build_nc(debug=False):
    import concourse.bacc as bacc
    import concourse.mybir as mybir
    import concourse.tile as tile

    BF16, F32 = mybir.dt.bfloat16, mybir.dt.float32
    FP8 = mybir.dt.float8e4
    EXP = mybir.ActivationFunctionType.Exp
    ADD = mybir.AluOpType.add
    MULT = mybir.AluOpType.mult
    DR = mybir.MatmulPerfMode.DoubleRow

    nc = bacc.Bacc(None)
    xT = nc.declare_dram_parameter("xT", [D, S], BF16, isOutput=False)
    wqk = nc.declare_dram_parameter("wqkT", [D, 512], BF16, isOutput=False)
    wv = nc.declare_dram_parameter("wvT", [D, 256], BF16, isOutput=False)
    wo = nc.declare_dram_parameter("woT", [256, D], BF16, isOutput=False)
    bqk = nc.declare_dram_parameter("bqk", [P, 4], F32, isOutput=False)
    bv = nc.declare_dram_parameter("bv", [1, 256], F32, isOutput=False)
    masks = nc.declare_dram_parameter("masks", [P, P], F32, isOutput=False)
    outT = nc.declare_dram_parameter("outT", [D, S], BF16, isOutput=True)
    if debug:
        dbg_q = nc.declare_dram_parameter("dbg_q", [P, 2, S], FP8, isOutput=True)
        dbg_k = nc.declare_dram_parameter("dbg_k", [P, 2, S], FP8, isOutput=True)
        dbg_v = nc.declare_dram_parameter(
            "dbg_v", [P, NKT_ALL, 4, HD + 1], BF16, isOutput=True)
        dbg_z = nc.declare_dram_parameter(
            "dbg_z", [P, 2, NSB, SB], BF16, isOutput=True)
        dbg_pr = nc.declare_dram_parameter("dbg_pr", [P, 2, SB], BF16, isOutput=True)
        dbg_sc = nc.declare_dram_parameter("dbg_sc", [P, 2, SB], F32, isOutput=True)
        dbg_wv = nc.declare_dram_parameter("dbg_wv", [P, SB], F32, isOutput=True)
        dbg_rb = nc.declare_dram_parameter("dbg_rb", [P, SB], F32, isOutput=True)

    with tile.TileContext(nc) as tc:
        with (
            tc.tile_pool(name="w", bufs=1) as wp,
            tc.tile_pool(name="x", bufs=2) as xp,
            tc.tile_pool(name="qk", bufs=1) as qkp,
            tc.tile_pool(name="pb", bufs=3) as pbp,
            tc.tile_pool(name="sm", bufs=2) as smp,
            tc.tile_pool(name="o", bufs=4) as op_,
        ):
            # ---- weights / constants (DMA order = need order) ----
            wqk_r = wqk[:].rearrange("(dk p) m -> p dk m", p=P)
            wqk_sb = wp.tile([P, NDK, 512], BF16)
            xT_r = xT[:].rearrange("(dk p) s -> p dk s", p=P)
            x_tiles = []
            x0 = xp.tile([P, NDK, SB], BF16, tag="x", name="x0")
            for d0 in range(0, NDK, 2):
                nc.sync.dma_start(wqk_sb[:, d0:d0 + 2, :], wqk_r[:, d0:d0 + 2, :])
                nc.sync.dma_start(x0[:, d0:d0 + 2, :], xT_r[:, d0:d0 + 2, 0:SB])
            x_tiles.append(x0)
            wv_sb = wp.tile([P, NDK, 256], BF16)
            nc.sync.dma_start(wv_sb[:], wv[:].rearrange("(dk p) m -> p dk m", p=P))
            bqk_sb = wp.tile([P, 4], F32)
            nc.sync.dma_start(bqk_sb[:], bqk[:])
            bv_sb = wp.tile([1, 256], F32)
            nc.sync.dma_start(bv_sb[:], bv[:])
            bv_bc = wp.tile([P, 256], F32)
            nc.gpsimd.partition_broadcast(bv_bc[:], bv_sb[:])
            mask_sb = wp.tile([P, P], F32)
            nc.sync.dma_start(mask_sb[:], masks[:])
            wo_sb = wp.tile([P, 2, D], BF16)
            nc.sync.dma_start(wo_sb[:], wo[:].rearrange("(k p) m -> p k m", p=P))

            # ---- persistent activations ----
            # q/k: fp8, partition 32h+r, free dims [t(2), s]; e = 32t + r
            qT8 = qkp.tile([P, 2, S], FP8, tag="qT8")
            kT8 = qkp.tile([P, 2, S], FP8, tag="kT8")
            # v: head slots 0..3 in order, each [v(0:64), one]
            v_sb = qkp.tile([P, NKT_ALL, 4, HD + 1], BF16, tag="v")
            nc.vector.memset(v_sb[:, :, :, HD:HD + 1], 1.0)
            zT_all = qkp.tile([P, 2, NSB, SB], BF16, tag="zT")

            with (
                tc.tile_pool(name="psS", bufs=2, space="PSUM") as psS,   # 4 banks
                tc.tile_pool(name="psW", bufs=1, space="PSUM") as psW,   # 2 banks
                tc.tile_pool(name="psG", bufs=2, space="PSUM") as psG,   # 2 banks
            ):
                def proj(blk):
                    x_sb = x_tiles[blk]
                    s0 = blk * SB
                    for g in range(4):  # qA, qB, kA, kB column groups
                        ps = psG.tile([P, SB], F32, tag="g")
                        for dk in range(NDK):
                            nc.tensor.matmul(
                                ps[:], wqk_sb[:, dk, 128 * g:128 * g + 128],
                                x_sb[:, dk, :],
                                start=(dk == 0), stop=(dk == NDK - 1))
                        dst = qT8 if g < 2 else kT8
                        nc.vector.tensor_scalar_add(
                            dst[:, g % 2, s0:s0 + SB], ps[:], bqk_sb[:, g:g + 1])
                    for st in range(4):
                        kt = blk * 4 + st
                        psv = psG.tile([P, 256], F32, tag="g")
                        for dk in range(NDK):
                            nc.tensor.matmul(
                                psv[:], x_sb[:, dk, st * P:(st + 1) * P],
                                wv_sb[:, dk, :],
                                start=(dk == 0), stop=(dk == NDK - 1))
                        nc.vector.tensor_tensor(
                            v_sb[:, kt, :, 0:HD],
                            psv[:].rearrange("p (m e) -> p m e", m=4),
                            bv_bc[:].rearrange("p (m e) -> p m e", m=4), ADD)

                def oproj(j):
                    for et in range(NDK):
                        po = psG.tile([P, SB], F32, tag="g")
                        for k2 in range(2):
                            nc.tensor.matmul(
                                po[:], wo_sb[:, k2, et * P:(et + 1) * P],
                                zT_all[:, k2, j, :],
                                start=(k2 == 0), stop=(k2 == 1))
                        ot = op_.tile([P, SB], BF16, tag="ot")
                        nc.vector.tensor_copy(ot[:], po[:])
                        nc.sync.dma_start(
                            outT[:][et * P:(et + 1) * P, j * SB:(j + 1) * SB],
                            ot[:])

                proj(0)
                for j in range(NSB):
                    if j + 1 < NSB:  # prefetch next x block
                        xn = xp.tile([P, NDK, SB], BF16, tag="x", name=f"x{j+1}")
                        nc.sync.dma_start(xn[:], xT_r[:, :, (j + 1) * SB:(j + 2) * SB])
                        x_tiles.append(xn)
                    NKT = 4 * (j + 1)
                    q0 = j * SB
                    for p in range(2):
                        wv0 = psW.tile([P, SB], F32, tag="wv0", name="wv0")
                        wv1 = psW.tile([P, SB], F32, tag="wv1", name="wv1")
                        for kt in range(NKT):
                            t = kt - 4 * j
                            c0 = 128 * t if t > 0 else 0
                            w = SB - c0
                            sc = psS.tile([P, 2, SB], F32, tag="sc")
                            for half in range(2):
                                h = 2 * p + half
                                nc.tensor.matmul(
                                    sc[:, half, c0:SB],
                                    kT8[32 * h:32 * h + 32, :, kt * P:(kt + 1) * P],
                                    qT8[32 * h:32 * h + 32, :, q0 + c0:q0 + SB],
                                    start=True, stop=True, perf_mode=DR,
                                    tile_position=(32 * h, 0))
                            if t >= 0:  # triangle mask on first 128 live cols
                                nc.vector.tensor_tensor(
                                    sc[:, :, c0:c0 + P], sc[:, :, c0:c0 + P],
                                    mask_sb[:, None, :].to_broadcast([P, 2, P]),
                                    ADD)
                            pr = pbp.tile([P, 2, SB], BF16, tag="pr")
                            nc.scalar.activation(
                                pr[:, :, c0:SB], sc[:, :, c0:SB], EXP)
                            if debug and j == 0 and p == 0 and kt == 0:
                                scc = smp.tile([P, 2, SB], F32, tag="scc")
                                nc.vector.tensor_copy(scc[:], sc[:])
                                nc.sync.dma_start(dbg_sc[:], scc[:])
                                nc.sync.dma_start(dbg_pr[:], pr[:])
                            nc.tensor.matmul(
                                wv0[0:HD + 1, c0:SB], v_sb[:, kt, 2 * p, :],
                                pr[:, 0, c0:SB],
                                start=(kt == 0), stop=(kt == NKT - 1))
                            nc.tensor.matmul(
                                wv1[0:HD + 1, c0:SB], v_sb[:, kt, 2 * p + 1, :],
                                pr[:, 1, c0:SB],
                                start=(kt == 0), stop=(kt == NKT - 1))
                        # normalize: den row -> [1,SB] tile, recip, broadcast,
                        # one mult (den copy shifts partition 64 -> 0)
                        for half, wvh in ((0, wv0), (1, wv1)):
                            dn = smp.tile([1, SB], F32, tag=f"dn{half}")
                            nc.vector.tensor_copy(dn[:], wvh[HD:HD + 1, :])
                            rcp = smp.tile([1, SB], F32, tag=f"rcp{half}")
                            nc.vector.reciprocal_approx_fast(rcp[:], dn[:])
                            rb = smp.tile([P, SB], F32, tag=f"rb{half}")
                            nc.gpsimd.partition_broadcast(rb[0:HD, :], rcp[:])
                            if half == 0:
                                nc.vector.tensor_tensor(
                                    zT_all[0:HD, p, j, :], wvh[0:HD, :],
                                    rb[0:HD, :], MULT)
                            else:
                                zt_o = smp.tile([HD, SB], BF16, tag="zt_o")
                                nc.vector.tensor_tensor(
                                    zt_o[:], wvh[0:HD, :], rb[0:HD, :], MULT)
                                nc.sync.dma_start(zT_all[HD:P, p, j, :], zt_o[:])
                            if debug and j == 0 and p == 0 and half == 0:
                                wvc = smp.tile([P, SB], F32, tag="wvc")
                                nc.vector.tensor_copy(wvc[:], wvh[:])
                                nc.sync.dma_start(dbg_wv[:], wvc[:])
                                nc.sync.dma_start(dbg_rb[:], rb[:])
                    if j + 1 < NSB:
                        proj(j + 1)
                    if j >= 1:
                        oproj(j - 1)
                oproj(NSB - 1)
                if debug:
                    nc.sync.dma_start(dbg_q[:], qT8[:])
                    nc.sync.dma_start(dbg_k[:], kT8[:])
                    nc.sync.dma_start(dbg_v[:], v_sb[:])
                    nc.sync.dma_start(dbg_z[:], zT_all[:])

    nc.compile()
    return nc


def _host_inputs(inputs, Wq, bq, Wk, bk, Wv, bv, Wo, bo):
    """Build the 8 per-core input maps."""
    import ml_dtypes
    bf16 = ml_dtypes.bfloat16
    scale = np.float32(1.0 / np.sqrt(HD))
    in_maps = []
    # within-tile causal triangle mask [128 k, 128 q]
    kk = np.arange(P)[:, None]
    qq = np.arange(P)[None, :]
    m = np.where(kk <= qq, 0.0, -1e30).astype(np.float32)
    for c in range(NCORES):
        b = c // 4
        hg = c % 4
        hs = slice(4 * hg, 4 * hg + 4)
        xTc = np.asarray(inputs[b], np.float32).T.astype(bf16)
        Wq_h = np.asarray(Wq[hs], np.float32) * scale   # [4, 64, D]
        Wk_h = np.asarray(Wk[hs], np.float32)
        bq_h = (np.asarray(bq[hs], np.float32) * scale)  # [4, 64]
        bk_h = np.asarray(bk[hs], np.float32)
        # column groups: partitions 32h+r; group t holds e = 32t + r
        cols = []
        bcols = []
        for W, bb in ((Wq_h, bq_h), (Wk_h, bk_h)):
            for t in range(2):
                cols.append(W[:, 32 * t:32 * t + 32, :].reshape(P, D).T)
                bcols.append(bb[:, 32 * t:32 * t + 32].reshape(P))
        wqkT = np.concatenate(cols, axis=1).astype(bf16)          # [D, 512]
        bqk_c = np.stack(bcols, axis=1).astype(np.float32)        # [128, 4]
        WvT = np.asarray(Wv[hs], np.float32)
        WvT = WvT.transpose(2, 0, 1).reshape(D, 256).astype(bf16)
        bv_c = np.asarray(bv[hs], np.float32).reshape(1, 256)
        woT = np.asarray(Wo, np.float32)[:, 256 * hg:256 * (hg + 1)].T.astype(bf16)
        in_maps.append({
            "xT": np.ascontiguousarray(xTc), "wqkT": np.ascontiguousarray(wqkT),
            "wvT": np.ascontiguousarray(WvT), "woT": np.ascontiguousarray(woT),
            "bqk": np.ascontiguousarray(bqk_c),
            "bv": np.ascontiguousarray(bv_c), "masks": m,
        })
    return in_maps


def kernel(inputs, Wq, bq, Wk, bk, Wv, bv, Wo, bo):
    from concourse.bass_utils import run_bass_kernel_spmd

    if "nc" not in _CACHE:
        _CACHE["nc"] = _build_nc()
    nc = _CACHE["nc"]
    in_maps = _host_inputs(inputs, Wq, bq, Wk, bk, Wv, bv, Wo, bo)
    res = run_bass_kernel_spmd(nc, in_maps, list(range(NCORES)))
    out = np.zeros((B, S, D), dtype=np.float32)
    for c in range(NCORES):
        out[c // 4] += np.asarray(res.results[c]["outT"], np.float32).T
    out += np.asarray(bo, np.float32)[None, None, :]
    return out


# revision 19
# speedup vs baseline: 1.3884x; 1.0527x over previous
"""Causal multi-head attention (B=2, S=2048, D=1024, H=16) on 8 TRN2 NeuronCores.

Sharding: core c handles batch b=c//4 and the 4 heads [4*(c%4), 4*(c%4)+4).
Each core computes its heads' Q/K/V projections, causal attention, and a
column-shard of the output projection; the host sums the 4 partials per batch
and adds bo.

Key layout/perf choices (v2):
  - all weights/activations bf16 except: score psums f32, q/k stored fp8e4m3
  - scores run as fp8 DoubleRow matmuls: q/k stored [32h+r, t, s] with the
    64-dim head split into two 32-row groups (t); the projection weight
    columns are pre-permuted on the host so the PSUM partition layout matches
    (bias-add stays partition-preserving). 2x PE throughput on scores.
  - diagonal k-tiles narrowed: score/exp/AV only touch q >= 128*t, the
    causal triangle mask is a single [128,128] additive tile.
  - AV stationary carries a ones-column; half 0 outputs psum rows 0..64
    (row 64 = denominator), half 1 uses [one, v] so its 65 output rows land
    at 63..127 (row 63 = denominator) -> z lands on its natural partitions,
    no cross-partition DMA.
  - normalization: reciprocal on the single denominator row, gpsimd
    partition-broadcast, one DVE mult into bf16 zT.
  - O-projection for block j is issued one block late (lowest priority) so
    it fills PE gaps; exp is the only Scalar-engine op (one act table).
"""

import numpy as np

B, S, D, H = 2, 2048, 1024, 16
HD = D // H  # 64
NCORES = 8
P = 128
SB = 512          # s/q block size
NSB = S // SB     # 4
NDK = D // P      # 8
NKT_ALL = S // P  # 16

_CACHE = {}


def _build_nc(debug=False):
    import concourse.bacc as bacc
    import concourse.mybir as mybir
    import concourse.tile as tile

    BF16, F32 = mybir.dt.bfloat16, mybir.dt.float32
    FP8 = mybir.dt.float8e4
    EXP = mybir.ActivationFunctionType.Exp
    ADD = mybir.AluOpType.add
    MULT = mybir.AluOpType.mult
    DR = mybir.MatmulPerfMode.DoubleRow

    nc = bacc.Bacc(None)
    xT = nc.declare_dram_parameter("xT", [D, S], BF16, isOutput=False)
    wqk = nc.declare_dram_parameter("wqkT", [D, 512], BF16, isOutput=False)
    wv = nc.declare_dram_parameter("wvT", [D, 256], BF16, isOutput=False)
    wo = nc.declare_dram_parameter("woT", [256, D], BF16, isOutput=False)
    bqk = nc.declare_dram_parameter("bqk", [P, 4], F32, isOutput=False)
    bv = nc.declare_dram_parameter("bv", [1, 256], F32, isOutput=False)
    masks = nc.declare_dram_parameter("masks", [P, P], F32, isOutput=False)
    outT = nc.declare_dram_parameter("outT", [D, S], BF16, isOutput=True)
    if debug:
        dbg_q = nc.declare_dram_parameter("dbg_q", [P, 2, S], FP8, isOutput=True)
        dbg_k = nc.declare_dram_parameter("dbg_k", [P, 2, S], FP8, isOutput=True)
        dbg_v = nc.declare_dram_parameter(
            "dbg_v", [P, NKT_ALL, 4, HD + 1], BF16, isOutput=True)
        dbg_z = nc.declare_dram_parameter(
            "dbg_z", [P, 2, NSB, SB], BF16, isOutput=True)
        dbg_pr = nc.declare_dram_parameter("dbg_pr", [P, 2, SB], BF16, isOutput=True)
        dbg_sc = nc.declare_dram_parameter("dbg_sc", [P, 2, SB], F32, isOutput=True)
        dbg_wv = nc.declare_dram_parameter("dbg_wv", [P, SB], F32, isOutput=True)
        dbg_rb = nc.declare_dram_parameter("dbg_rb", [P, SB], F32, isOutput=True)

    with tile.TileContext(nc) as tc:
        with (
            tc.tile_pool(name="w", bufs=1) as wp,
            tc.tile_pool(name="x", bufs=2) as xp,
            tc.tile_pool(name="qk", bufs=1) as qkp,
            tc.tile_pool(name="pb", bufs=3) as pbp,
            tc.tile_pool(name="sm", bufs=2) as smp,
            tc.tile_pool(name="o", bufs=4) as op_,
        ):
            # ---- weights / constants (DMA order = need order) ----
            wqk_r = wqk[:].rearrange("(dk p) m -> p dk m", p=P)
            wqk_sb = wp.tile([P, NDK, 512], BF16)
            xT_r = xT[:].rearrange("(dk p) s -> p dk s", p=P)
            x_tiles = []
            x0 = xp.tile([P, NDK, SB], BF16, tag="x", name="x0")
            # spread initial DMA issues across idle engine queues so the
            # first projection matmul isn't gated on a serial Sync queue
            nc.sync.dma_start(wqk_sb[:, 0:2, :], wqk_r[:, 0:2, :])
            nc.sync.dma_start(x0[:, 0:2, :], xT_r[:, 0:2, 0:SB])
            for d0, eng in ((2, nc.gpsimd), (4, nc.scalar), (6, nc.gpsimd)):
                eng.dma_start(wqk_sb[:, d0:d0 + 2, :], wqk_r[:, d0:d0 + 2, :])
                eng.dma_start(x0[:, d0:d0 + 2, :], xT_r[:, d0:d0 + 2, 0:SB])
            x_tiles.append(x0)
            wv_sb = wp.tile([P, NDK, 256], BF16)
            nc.gpsimd.dma_start(wv_sb[:], wv[:].rearrange("(dk p) m -> p dk m", p=P))
            bqk_sb = wp.tile([P, 4], F32)
            nc.scalar.dma_start(bqk_sb[:], bqk[:])
            bv_sb = wp.tile([1, 256], F32)
            nc.scalar.dma_start(bv_sb[:], bv[:])
            bv_bc = wp.tile([P, 256], F32)
            nc.gpsimd.partition_broadcast(bv_bc[:], bv_sb[:])
            mask_sb = wp.tile([P, P], F32)
            nc.scalar.dma_start(mask_sb[:], masks[:])
            wo_sb = wp.tile([P, 2, D], BF16)
            nc.sync.dma_start(wo_sb[:], wo[:].rearrange("(k p) m -> p k m", p=P))

            # ---- persistent activations ----
            # q/k: fp8, partition 32h+r, free dims [t(2), s]; e = 32t + r
            qT8 = qkp.tile([P, 2, S], FP8, tag="qT8")
            kT8 = qkp.tile([P, 2, S], FP8, tag="kT8")
            # v: head slots 0..3 in order, each [v(0:64), one]
            v_sb = qkp.tile([P, NKT_ALL, 4, HD + 1], BF16, tag="v")
            nc.vector.memset(v_sb[:, :, :, HD:HD + 1], 1.0)
            zT_all = qkp.tile([P, 2, NSB, SB], BF16, tag="zT")

            with (
                tc.tile_pool(name="psS", bufs=2, space="PSUM") as psS,   # 4 banks
                tc.tile_pool(name="psW", bufs=1, space="PSUM") as psW,   # 2 banks
                tc.tile_pool(name="psG", bufs=2, space="PSUM") as psG,   # 2 banks
            ):
                def proj(blk):
                    x_sb = x_tiles[blk]
                    s0 = blk * SB
                    for g in range(4):  # qA, qB, kA, kB column groups
                        ps = psG.tile([P, SB], F32, tag="g")
                        for dk in range(NDK):
                            nc.tensor.matmul(
                                ps[:], wqk_sb[:, dk, 128 * g:128 * g + 128],
                                x_sb[:, dk, :],
                                start=(dk == 0), stop=(dk == NDK - 1))
                        dst = qT8 if g < 2 else kT8
                        nc.vector.tensor_scalar_add(
                            dst[:, g % 2, s0:s0 + SB], ps[:], bqk_sb[:, g:g + 1])
                    for st in range(4):
                        kt = blk * 4 + st
                        psv = psG.tile([P, 256], F32, tag="g")
                        for dk in range(NDK):
                            nc.tensor.matmul(
                                psv[:], x_sb[:, dk, st * P:(st + 1) * P],
                                wv_sb[:, dk, :],
                                start=(dk == 0), stop=(dk == NDK - 1))
                        nc.vector.tensor_tensor(
                            v_sb[:, kt, :, 0:HD],
                            psv[:].rearrange("p (m e) -> p m e", m=4),
                            bv_bc[:].rearrange("p (m e) -> p m e", m=4), ADD)

                def oproj(j):
                    for et in range(NDK):
                        po = psG.tile([P, SB], F32, tag="g")
                        for k2 in range(2):
                            nc.tensor.matmul(
                                po[:], wo_sb[:, k2, et * P:(et + 1) * P],
                                zT_all[:, k2, j, :],
                                start=(k2 == 0), stop=(k2 == 1))
                        ot = op_.tile([P, SB], BF16, tag="ot")
                        nc.vector.tensor_copy(ot[:], po[:])
                        nc.sync.dma_start(
                            outT[:][et * P:(et + 1) * P, j * SB:(j + 1) * SB],
                            ot[:])

                proj(0)
                for j in range(NSB):
                    if j + 1 < NSB:  # prefetch next x block
                        xn = xp.tile([P, NDK, SB], BF16, tag="x", name=f"x{j+1}")
                        nc.sync.dma_start(xn[:], xT_r[:, :, (j + 1) * SB:(j + 2) * SB])
                        x_tiles.append(xn)
                    NKT = 4 * (j + 1)
                    q0 = j * SB
                    for p in range(2):
                        wv0 = psW.tile([P, SB], F32, tag="wv0", name="wv0")
                        wv1 = psW.tile([P, SB], F32, tag="wv1", name="wv1")
                        for kt in range(NKT):
                            t = kt - 4 * j
                            c0 = 128 * t if t > 0 else 0
                            w = SB - c0
                            sc = psS.tile([P, 2, SB], F32, tag="sc")
                            for half in range(2):
                                h = 2 * p + half
                                nc.tensor.matmul(
                                    sc[:, half, c0:SB],
                                    kT8[32 * h:32 * h + 32, :, kt * P:(kt + 1) * P],
                                    qT8[32 * h:32 * h + 32, :, q0 + c0:q0 + SB],
                                    start=True, stop=True, perf_mode=DR,
                                    tile_position=(32 * h, 0))
                            if t >= 0:  # triangle mask on first 128 live cols
                                nc.vector.tensor_tensor(
                                    sc[:, :, c0:c0 + P], sc[:, :, c0:c0 + P],
                                    mask_sb[:, None, :].to_broadcast([P, 2, P]),
                                    ADD)
                            pr = pbp.tile([P, 2, SB], BF16, tag="pr")
                            nc.scalar.activation(
                                pr[:, :, c0:SB], sc[:, :, c0:SB], EXP)
                            if debug and j == 0 and p == 0 and kt == 0:
                                scc = smp.tile([P, 2, SB], F32, tag="scc")
                                nc.vector.tensor_copy(scc[:], sc[:])
                                nc.sync.dma_start(dbg_sc[:], scc[:])
                                nc.sync.dma_start(dbg_pr[:], pr[:])
                            nc.tensor.matmul(
                                wv0[0:HD + 1, c0:SB], v_sb[:, kt, 2 * p, :],
                                pr[:, 0, c0:SB],
                                start=(kt == 0), stop=(kt == NKT - 1))
                            nc.tensor.matmul(
                                wv1[0:HD + 1, c0:SB], v_sb[:, kt, 2 * p + 1, :],
                                pr[:, 1, c0:SB],
                                start=(kt == 0), stop=(kt == NKT - 1))
                        # normalize: den row -> [1,SB] tile (copy shifts
                        # partition 64 -> 0), recip, broadcast, one mult
                        for half, wvh in ((0, wv0), (1, wv1)):
                            dn = smp.tile([1, SB], F32, tag=f"dn{half}")
                            nc.vector.tensor_copy(dn[:], wvh[HD:HD + 1, :])
                            rcp = smp.tile([1, SB], F32, tag=f"rcp{half}")
                            nc.vector.reciprocal_approx_fast(rcp[:], dn[:])
                            rb = smp.tile([P, SB], F32, tag=f"rb{half}")
                            nc.gpsimd.partition_broadcast(rb[0:HD, :], rcp[:])
                            if half == 0:
                                nc.vector.tensor_tensor(
                                    zT_all[0:HD, p, j, :], wvh[0:HD, :],
                                    rb[0:HD, :], MULT)
                            else:
                                zt_o = smp.tile([HD, SB], BF16, tag="zt_o")
                                nc.vector.tensor_tensor(
                                    zt_o[:], wvh[0:HD, :], rb[0:HD, :], MULT)
                                nc.sync.dma_start(zT_all[HD:P, p, j, :], zt_o[:])
                            if debug and j == 0 and p == 0 and half == 0:
                                wvc = smp.tile([P, SB], F32, tag="wvc")
                                nc.vector.tensor_copy(wvc[:], wvh[:])
                                nc.sync.dma_start(dbg_wv[:], wvc[:])
                                nc.sync.dma_start(dbg_rb[:], rb[:])
                    if j + 1 < NSB:
                        proj(j + 1)
                # all O-projections issued last: they fill PE idle in the
                # exp-bound final attention window
                for j in range(NSB):
                    oproj(j)
                if debug:
                    nc.sync.dma_start(dbg_q[:], qT8[:])
                    nc.sync.dma_start(dbg_k[:], kT8[:])
                    nc.sync.dma_start(dbg_v[:], v_sb[:])
                    nc.sync.dma_start(dbg_z[:], zT_all[:])

    nc.compile()
    return nc


def _host_inputs(inputs, Wq, bq, Wk, bk, Wv, bv, Wo, bo):
    """Build the 8 per-core input maps."""
    import ml_dtypes
    bf16 = ml_dtypes.bfloat16
    scale = np.float32(1.0 / np.sqrt(HD))
    in_maps = []
    # within-tile causal triangle mask [128 k, 128 q]
    kk = np.arange(P)[:, None]
    qq = np.arange(P)[None, :]
    m = np.where(kk <= qq, 0.0, -1e30).astype(np.float32)
    for c in range(NCORES):
        b = c // 4
        hg = c % 4
        hs = slice(4 * hg, 4 * hg + 4)
        xTc = np.asarray(inputs[b], np.float32).T.astype(bf16)
        Wq_h = np.asarray(Wq[hs], np.float32) * scale   # [4, 64, D]
        Wk_h = np.asarray(Wk[hs], np.float32)
        bq_h = (np.asarray(bq[hs], np.float32) * scale)  # [4, 64]
        bk_h = np.asarray(bk[hs], np.float32)
        # column groups: partitions 32h+r; group t holds e = 32t + r
        cols = []
        bcols = []
        for W, bb in ((Wq_h, bq_h), (Wk_h, bk_h)):
            for t in range(2):
                cols.append(W[:, 32 * t:32 * t + 32, :].reshape(P, D).T)
                bcols.append(bb[:, 32 * t:32 * t + 32].reshape(P))
        wqkT = np.concatenate(cols, axis=1).astype(bf16)          # [D, 512]
        bqk_c = np.stack(bcols, axis=1).astype(np.float32)        # [128, 4]
        WvT = np.asarray(Wv[hs], np.float32)
        WvT = WvT.transpose(2, 0, 1).reshape(D, 256).astype(bf16)
        bv_c = np.asarray(bv[hs], np.float32).reshape(1, 256)
        woT = np.asarray(Wo, np.float32)[:, 256 * hg:256 * (hg + 1)].T.astype(bf16)
        in_maps.append({
            "xT": np.ascontiguousarray(xTc), "wqkT": np.ascontiguousarray(wqkT),
            "wvT": np.ascontiguousarray(WvT), "woT": np.ascontiguousarray(woT),
            "bqk": np.ascontiguousarray(bqk_c),
            "bv": np.ascontiguousarray(bv_c), "masks": m,
        })
    return in_maps


def kernel(inputs, Wq, bq, Wk, bk, Wv, bv, Wo, bo):
    from concourse.bass_utils import run_bass_kernel_spmd

    if "nc" not in _CACHE:
        _CACHE["nc"] = _build_nc()
    nc = _CACHE["nc"]
    in_maps = _host_inputs(inputs, Wq, bq, Wk, bk, Wv, bv, Wo, bo)
    res = run_bass_kernel_spmd(nc, in_maps, list(range(NCORES)))
    out = np.zeros((B, S, D), dtype=np.float32)
    for c in range(NCORES):
        out[c // 4] += np.asarray(res.results[c]["outT"], np.float32).T
    out += np.asarray(bo, np.float32)[None, None, :]
    return out
